# revision 21
# baseline (speedup 1.0000x reference)
"""Bass/Tile Trainium2 kernel for nn_Attention (B=4, T=4096, C=256), 8 cores.

Sharding: core = (batch b, query-half h). Each core computes the full K/V
projections for its batch and attention output for its 2048 query rows.

Key compaction: the 0/1 key mask keeps ~50% of keys. The host gathers the
valid key columns of x^T per batch (padded with zeros to TK), so the device
only projects/attends over TK=2176 keys instead of T=4096 — softmax over
the compacted key set is exact (the torch +1.0-on-valid-keys quirk is a
uniform shift that cancels; padding keys have v=0 and a zeroed ones-column
entry so they drop out of both softmax sums). Falls back to a full-T build
if a batch ever has more than TK valid keys.

Layout strategy (all matmuls bf16, fp32 PSUM accumulation):
  - Host pre-transposes x to x^T [C, T]; projections contract C on
    partitions. k^T/q^T come out feature-major, so the score matmul
    produces scoresT [keys j on partitions, queries q on free dim].
  - Softmax needs no max-subtraction (scores are O(1); exp cannot
    overflow fp32) and no partition reductions.
  - V gets a column of ones appended: out[q, 256] accumulates the
    softmax denominator for free. Final: out[:, :256] * (1/out[:, 256]).
  - Main loop is software-pipelined per key block: PE does the two score
    matmuls for block jb+1 and then the four out-matmuls for block jb,
    so ACT's exp (~720 ns/tile) hides behind PE work.
"""

import numpy as np
import ml_dtypes

import concourse.bacc as bacc
import concourse.mybir as mybir
import concourse.tile as tile
from concourse.bass_utils import run_bass_kernel_spmd

B, T, C = 4, 4096, 256
NCORES = 8
HALVES = NCORES // B          # 2 query-halves per batch
TQ = T // HALVES              # 2048 query rows per core
PB = 128                      # partition block
NCCH = C // PB                # 2 contraction chunks of 128
TK = 2176                     # compacted+padded key count (17 blocks of 128)
SBW = 512                     # query superblock width
NSB = TQ // SBW               # 4 superblocks per core
NQB = SBW // PB               # 4 query 128-blocks per superblock
VW = C + 1                    # v tile width incl. ones column
SCALE = float(C) ** -0.5
BF16 = mybir.dt.bfloat16
F32 = mybir.dt.float32
FP8 = mybir.dt.float8e4
VWP = 272                     # fp8 va block pitch (16B-aligned for DoubleRow)
FP8_EXP_BIAS = -6.0           # exp shift so p fits fp8e4m3 range; cancels in softmax


def _emit(tc, out, xt, xq, mt, wv, mb, tk, mode="full", fp8=False):
    nc = tc.nc
    import contextlib
    njb = tk // PB            # key blocks

    with contextlib.ExitStack() as ctx:
        persist = ctx.enter_context(tc.tile_pool(name="persist", bufs=1))
        # Persistent SBUF tensors; c-chunks laid side by side on the free dim.
        xt_sb = persist.tile([PB, NCCH * tk], BF16)   # x^T  (compacted keys)
        xq_sb = persist.tile([PB, NCCH * TQ], BF16)   # x^T  (this core's half)
        mt_sb = persist.tile([PB, NCCH * C], BF16)    # (Wq^T Wk) fused weight
        wv_sb = persist.tile([PB, NCCH * C], BF16)
        xm_sb = persist.tile([PB, NCCH * TQ], BF16)   # M^T x_q  (query-side)
        assert not fp8, "fp8 path removed (accuracy: p+v fp8 exceeds tolerance)"
        vw = VW
        va_sb = persist.tile([PB, njb * vw], BF16)    # masked v + masked ones col
        mb_sb = persist.tile([PB, njb], F32)          # 0/1 mask, [j in block, jb]

        # Few, large, descriptor-friendly DMAs spread across the three
        # DMA-capable queues (sync/scalar HWDGE, gpsimd SWDGE). xq and
        # weights land first so the q projection starts while xt streams.
        w2 = lambda w: w.rearrange("(n p) c -> p n c", p=PB)
        s3 = lambda t, n: t.rearrange("p (n c) -> p n c", n=n)
        nc.scalar.dma_start(s3(mt_sb[:], NCCH), w2(mt))
        nc.gpsimd.dma_start(s3(wv_sb[:], NCCH), w2(wv))
        nc.gpsimd.dma_start(mb_sb[:], mb)
        xq3 = xq.rearrange("(n p) t -> p n t", p=PB)
        # xq in pieces so the q-projection's first blocks start early
        for lo, hi in ((0, 512), (512, 1024), (1024, TQ)):
            dst = xq_sb[:].rearrange("p (n t) -> p n t", n=NCCH)[:, :, lo:hi]
            nc.sync.dma_start(dst, xq3[:, :, lo:hi])
        H = tk // 2
        nc.sync.dma_start(xt_sb[:, 0:H], xt[0:PB, 0:H])
        nc.scalar.dma_start(xt_sb[:, tk:tk + H], xt[PB:2 * PB, 0:H])
        nc.sync.dma_start(xt_sb[:, H:tk], xt[0:PB, H:tk])
        nc.scalar.dma_start(xt_sb[:, tk + H:2 * tk], xt[PB:2 * PB, H:tk])

        # masked ones column: va[:, jb*vw + C] = mask01[:, jb]
        va_ones = va_sb[:].rearrange("p (j e) -> p j e", e=vw)[:, :, C:C + 1]
        nc.vector.tensor_copy(va_ones, mb_sb[:].rearrange("p (j e) -> p j e", e=1))

        # ---- projections ----
        # The f32 PSUM -> bf16 SBUF copies are the proj-phase bottleneck;
        # round-robin them across DVE and ACT to balance the two engines.
        cp_engs = (nc.vector.tensor_copy, nc.scalar.copy)
        cp_i = [0]

        def cp(dst, src, eng=None):
            (eng or cp_engs[cp_i[0] % 2])(dst, src)
            cp_i[0] += 1

        with tc.tile_pool(name="proj_psum", bufs=2, space="PSUM") as pp:
            # xm[c, q] = sum_c' Mt[c', c] xq[c', q]: one fused projection
            # replaces both the q- and k-projections (M = Wq^T Wk from host);
            # the score matmul's stationary side reads raw xt from SBUF.
            for w_sb, x_src, x_w, dst in (
                (mt_sb, xq_sb, TQ, xm_sb),
            ):
                nblk = x_w // 512
                widths = [512] * nblk + ([x_w - nblk * 512] if x_w % 512 else [])
                off = 0
                for wdt in widths:
                    for dc in range(NCCH):
                        ps = pp.tile([PB, 512], F32, tag="proj", name="proj_ps")
                        for cc in range(NCCH):
                            nc.tensor.matmul(
                                ps[:, 0:wdt],
                                lhsT=w_sb[:, cc * C + dc * PB: cc * C + (dc + 1) * PB],
                                rhs=x_src[:, cc * x_w + off: cc * x_w + off + wdt],
                                start=(cc == 0),
                                stop=(cc == NCCH - 1),
                            )
                        cp(dst[:, dc * x_w + off: dc * x_w + off + wdt],
                           ps[:, 0:wdt])
                    off += wdt
            # v[t, d]: lhsT = x^T chunk [c, t-block], rhs = W^T chunk [c, d].
            # xt is host-compacted (only valid keys, zero pad), so v pad rows
            # are 0 and the ones column carries the pad mask. Two key blocks
            # share one PSUM bank so each copy moves 512 columns.
            for jp in range(0, njb, 2):
                pair = min(2, njb - jp)
                ps = pp.tile([PB, 512], F32, tag="projv", name="projv_ps")
                for j in range(pair):
                    for cc in range(NCCH):
                        nc.tensor.matmul(
                            ps[:, j * C:(j + 1) * C],
                            lhsT=xt_sb[:, cc * tk + (jp + j) * PB:
                                       cc * tk + (jp + j + 1) * PB],
                            rhs=wv_sb[:, cc * C:(cc + 1) * C],
                            start=(cc == 0),
                            stop=(cc == NCCH - 1),
                        )
                dstv = va_sb[:, jp * vw:(jp + pair) * vw].rearrange(
                    "p (j e) -> p j e", e=vw)[:, :, 0:C]
                srcv = ps[:, 0:pair * C].rearrange("p (j e) -> p j e", e=C)
                # DVE only (GPSIMD cannot read PSUM): keeps ACT's FIFO clear
                # for the first exps of the main loop.
                cp(dstv, srcv, eng=nc.vector.tensor_copy)

        # ---- attention main loop ----
        scp = ctx.enter_context(tc.tile_pool(name="sc_psum", bufs=2, space="PSUM"))
        op = ctx.enter_context(tc.tile_pool(name="o_psum", bufs=1, space="PSUM"))
        ppool = ctx.enter_context(tc.tile_pool(name="p_pool", bufs=4))
        fin = ctx.enter_context(tc.tile_pool(name="fin", bufs=3))

        if mode == "projonly":
            os_t = fin.tile([PB, C], F32, tag="os", name="os_t")
            nc.vector.tensor_copy(os_t, xm_sb[:, 0:C])
            nc.sync.dma_start(out[0:PB, :], os_t)
            return
        if mode in ("noscores", "mmonly"):
            p_static = persist.tile([PB, 4 * SBW], BF16, name="p_static")
            nc.vector.memset(p_static[:], 1.0)
        if mode == "mmonly":
            # PE-pure benchmark: the real matmul sequence with no ACT/DVE
            # dependencies. Measures the raw HW MM+LDW stream rate.
            nc.vector.memset(va_sb[:], 0.5)
            nc.vector.memset(xm_sb[:], 0.5)
            for sb in range(NSB):
                op_tiles = [op.tile([PB, VW], F32, tag=f"o{qb}", name=f"opsum{qb}")
                            for qb in range(NQB)]
                for jb in range(njb):
                    ps = scp.tile([PB, 2 * SBW], F32, tag="sc", name="sc_ps")
                    for cc in range(NCCH):
                        nc.tensor.matmul(
                            ps[:, 0:SBW],
                            lhsT=xt_sb[:, cc * tk + jb * PB: cc * tk + (jb + 1) * PB],
                            rhs=xm_sb[:, cc * TQ + sb * SBW: cc * TQ + (sb + 1) * SBW],
                            start=(cc == 0),
                            stop=(cc == NCCH - 1),
                        )
                    for qb in range(NQB):
                        nc.tensor.matmul(
                            op_tiles[qb],
                            lhsT=p_static[:, (jb % 4) * SBW + qb * PB:
                                          (jb % 4) * SBW + (qb + 1) * PB],
                            rhs=va_sb[:, jb * VW:(jb + 1) * VW],
                            start=(jb == 0),
                            stop=(jb == njb - 1),
                        )
                os_t = fin.tile([PB, C], F32, tag="os", name="os_t")
                nc.vector.tensor_copy(os_t, op_tiles[0][:, 0:C])
                nc.sync.dma_start(out[sb * PB:(sb + 1) * PB, :], os_t)
            return

        for sb in range(NSB):
            if mode == "noout":
                op_tiles = None
            else:
                op_tiles = [op.tile([PB, VW], F32, tag=f"o{qb}", name=f"opsum{qb}")
                            for qb in range(NQB)]
            p_tiles = {}
            NG = (njb + 1) // 2       # score groups: pairs of key blocks

            def emit_scores_group(g, sb=sb, p_tiles=p_tiles):
                # two key blocks share one 2-bank PSUM tile and ONE exp
                # instruction - halves ACT instruction count and its fixed
                # per-instruction overhead.
                blocks = [2 * g] if 2 * g + 1 >= njb else [2 * g, 2 * g + 1]
                wide = len(blocks) * SBW
                ps = scp.tile([PB, 2 * SBW], F32, tag="sc", name="sc_ps")
                for bi, jb in enumerate(blocks):
                    for cc in range(NCCH):
                        nc.tensor.matmul(
                            ps[:, bi * SBW:(bi + 1) * SBW],
                            lhsT=xt_sb[:, cc * tk + jb * PB: cc * tk + (jb + 1) * PB],
                            rhs=xm_sb[:, cc * TQ + sb * SBW: cc * TQ + (sb + 1) * SBW],
                            start=(cc == 0),
                            stop=(cc == NCCH - 1),
                        )
                pt = ppool.tile([PB, 2 * SBW], BF16, tag="p", name="p_t")
                nc.scalar.activation(
                    pt[:, 0:wide], ps[:, 0:wide],
                    mybir.ActivationFunctionType.Exp, scale=SCALE)
                p_tiles[g] = pt

            def p_of(jb, p_tiles=p_tiles):
                return p_tiles[jb // 2], (jb % 2) * SBW

            def emit_out(jb, op_tiles=op_tiles, p_tiles=p_tiles):
                pt, off = p_of(jb)
                for qb in range(NQB):
                    nc.tensor.matmul(
                        op_tiles[qb],
                        lhsT=pt[:, off + qb * PB: off + (qb + 1) * PB],
                        rhs=va_sb[:, jb * VW:(jb + 1) * VW],
                        start=(jb == 0),
                        stop=(jb == njb - 1),
                    )
                if jb % 2 == 1 or jb == njb - 1:
                    p_tiles.pop(jb // 2)

            if mode == "noout":
                for g in range(NG):
                    emit_scores_group(g)
                    p_tiles.pop(g)
            elif mode == "noscores":
                for jb in range(njb):
                    for qb in range(NQB):
                        nc.tensor.matmul(
                            op_tiles[qb],
                            lhsT=p_static[:, (jb % 4) * SBW + qb * PB:
                                          (jb % 4) * SBW + (qb + 1) * PB],
                            rhs=va_sb[:, jb * VW:(jb + 1) * VW],
                            start=(jb == 0),
                            stop=(jb == njb - 1),
                        )
            elif sb < NSB - 1:
                # two score-groups (4 key blocks) of lookahead: exp(g) has
                # ~two group durations of PE slack before out(2g) needs it.
                emit_scores_group(0)
                emit_scores_group(1)
                for jb in range(njb):
                    if jb % 2 == 0 and jb // 2 + 2 < NG:
                        emit_scores_group(jb // 2 + 2)
                    emit_out(jb)
                os_t = fin.tile([PB, NQB * C], F32, tag="os", name="os_t")
                for qb in range(NQB):
                    rec = fin.tile([PB, 1], F32, tag="rec", name="rec_t")
                    nc.vector.reciprocal(rec, op_tiles[qb][:, C:C + 1])
                    nc.vector.tensor_scalar_mul(
                        os_t[:, qb * C:(qb + 1) * C], op_tiles[qb][:, 0:C], rec)
                # keep outputs off the sync/scalar queues that carry the
                # next For_i iteration's input DMAs.
                dma_eng = nc.gpsimd if sb % 2 == 0 else nc.scalar
                dma_eng.dma_start(
                    out[sb * SBW:(sb + 1) * SBW, :].rearrange("(q p) c -> p q c", p=PB),
                    os_t[:].rearrange("p (q c) -> p q c", q=NQB))
            else:
                # Final superblock: last TG key blocks are emitted qb-grouped
                # so each out-psum tile finishes early and its normalization +
                # store overlap the remaining PE work. ACT (done with exps)
                # shares the normalization muls.
                TG = 3
                emit_scores_group(0)
                emit_scores_group(1)
                ng_emitted = [2]
                os_t = fin.tile([PB, NQB * C], F32, tag="os", name="os_t")
                for jb in range(njb - TG):
                    if jb % 2 == 0:
                        while ng_emitted[0] < min(jb // 2 + 3, NG):
                            emit_scores_group(ng_emitted[0])
                            ng_emitted[0] += 1
                    emit_out(jb)
                while ng_emitted[0] < NG:
                    emit_scores_group(ng_emitted[0])
                    ng_emitted[0] += 1
                for qb in range(NQB):
                    for jb in range(njb - TG, njb):
                        pt, off = p_of(jb)
                        nc.tensor.matmul(
                            op_tiles[qb],
                            lhsT=pt[:, off + qb * PB: off + (qb + 1) * PB],
                            rhs=va_sb[:, jb * VW:(jb + 1) * VW],
                            start=False,
                            stop=(jb == njb - 1),
                        )
                    rec = fin.tile([PB, 1], F32, tag="rec", name="rec_t")
                    nc.vector.reciprocal(rec, op_tiles[qb][:, C:C + 1])
                    osq = os_t[:, qb * C:(qb + 1) * C]
                    if qb % 2 == 1:
                        nc.scalar.activation(
                            osq, op_tiles[qb][:, 0:C],
                            mybir.ActivationFunctionType.Copy, scale=rec[:])
                    else:
                        nc.vector.tensor_scalar_mul(
                            osq, op_tiles[qb][:, 0:C], rec)
                    dma_eng = nc.gpsimd if qb % 2 == 0 else nc.scalar
                    dma_eng.dma_start(
                        out[sb * SBW + qb * PB: sb * SBW + (qb + 1) * PB, :],
                        osq)
                for g in range((njb - TG) // 2, NG):
                    p_tiles.pop(g, None)
            if mode == "noout":
                os_t = fin.tile([PB, C], F32, tag="os", name="os_t")
                nc.vector.tensor_copy(os_t, xm_sb[:, sb * C:(sb + 1) * C])
                nc.sync.dma_start(out[sb * PB:(sb + 1) * PB, :], os_t)
                continue
            if mode == "noscores":
                os_t = fin.tile([PB, NQB * C], F32, tag="os", name="os_t")
                for qb in range(NQB):
                    rec = fin.tile([PB, 1], F32, tag="rec", name="rec_t")
                    nc.vector.reciprocal(rec, op_tiles[qb][:, C:C + 1])
                    nc.vector.tensor_scalar_mul(
                        os_t[:, qb * C:(qb + 1) * C], op_tiles[qb][:, 0:C], rec)
                dma_eng = nc.gpsimd if sb % 2 == 0 else nc.scalar
                dma_eng.dma_start(
                    out[sb * SBW:(sb + 1) * SBW, :].rearrange("(q p) c -> p q c", p=PB),
                    os_t[:].rearrange("p (q c) -> p q c", q=NQB))


def build_nc(reps=1, loop_n=0, mode="full", fp8=False, tk=TK):
    nc = bacc.Bacc("TRN2", target_bir_lowering=False, debug=False)
    xt = nc.dram_tensor("xt", [C, tk], BF16, kind="ExternalInput").ap()
    xq = nc.dram_tensor("xq", [C, TQ], BF16, kind="ExternalInput").ap()
    mt = nc.dram_tensor("mt", [C, C], BF16, kind="ExternalInput").ap()
    wv = nc.dram_tensor("wv", [C, C], BF16, kind="ExternalInput").ap()
    mb = nc.dram_tensor("mb", [PB, tk // PB], F32, kind="ExternalInput").ap()
    out = nc.dram_tensor("out", [TQ, C], F32, kind="ExternalOutput").ap()
    with tile.TileContext(nc) as tc:
        if loop_n:
            with tc.For_i(0, loop_n, 1, hint_engines=(mybir.EngineType.PE,)):
                _emit(tc, out, xt, xq, mt, wv, mb, tk, mode=mode, fp8=fp8)
        else:
            for _ in range(reps):
                _emit(tc, out, xt, xq, mt, wv, mb, tk, mode=mode, fp8=fp8)
    nc.compile()
    return nc


_CACHE = {}


def _get_nc(tk=TK):
    key = ("nc", tk)
    if key not in _CACHE:
        _CACHE[key] = build_nc(tk=tk)
    return _CACHE[key]


def make_in_maps(x, mask, tk=None):
    bf = ml_dtypes.bfloat16
    x = np.asarray(x, dtype=np.float32)
    m = np.asarray(mask) != 0                                    # [B, T]
    counts = m.sum(axis=1)
    if tk is None:
        tk = TK if counts.max() <= TK else T                     # fallback: no compaction
    xt_all = np.ascontiguousarray(x.transpose(0, 2, 1)).astype(bf)  # [B, C, T]
    maps = []
    xtc_all, mbc_all = [], []
    for b in range(B):
        idx = np.nonzero(m[b])[0]
        nv = len(idx)
        xtc = np.zeros((C, tk), dtype=bf)
        xtc[:, :nv] = xt_all[b][:, idx]
        mbc = np.zeros(tk, dtype=np.float32)
        mbc[:nv] = 1.0
        xtc_all.append(xtc)
        mbc_all.append(np.ascontiguousarray(mbc.reshape(tk // PB, PB).T))
    for core in range(NCORES):
        b, h = divmod(core, HALVES)
        maps.append({
            "xt": xtc_all[b],
            "xq": np.ascontiguousarray(xt_all[b][:, h * TQ:(h + 1) * TQ]),
            "mb": mbc_all[b],
        })
    return maps, tk


def make_wt_maps(Wk, Wq, Wv):
    bf = ml_dtypes.bfloat16
    wq32 = np.asarray(Wq, dtype=np.float32)
    wk32 = np.asarray(Wk, dtype=np.float32)
    # scoresT[k, q] = sum_c xt[c,k] xm[c,q], xm = Mt^T xq, Mt[c',c] = (Wq^T Wk)[c',c]
    mt = np.ascontiguousarray(wq32.T @ wk32).astype(bf)
    wvt = np.ascontiguousarray(np.asarray(Wv, dtype=np.float32).T).astype(bf)
    return {"mt": mt, "wv": wvt}


def kernel(x, mask, Wk, Wq, Wv):
    in_maps, tk = make_in_maps(x, mask)
    wts = make_wt_maps(Wk, Wq, Wv)
    for m in in_maps:
        m.update(wts)
    res = run_bass_kernel_spmd(_get_nc(tk), in_maps, list(range(NCORES)))
    out = np.empty((B, T, C), np.float32)
    for core in range(NCORES):
        b, h = divmod(core, HALVES)
        out[b, h * TQ:(h + 1) * TQ, :] = res.results[core]["out"]
    return out
